# revision 1
# baseline (speedup 1.0000x reference)
"""Trainium2 Bass kernel for nn_DirectionalScan (2D directional diagonal-SSM + projection).

Math: for each of two directions (scan over h, scan over w),
    y[t] = sum_n Cm*Bm * sum_{u<=t} A^(t-u) x[u]  + D_skip*x[t]
then out = (y_h + y_v) @ Wp.T + b_proj.

Device decomposition (validated in fp64/fp32 numpy): chunked SSM with chunk Q=16,
all heavy work on the PE as matmuls:
  - intra-chunk causal Toeplitz (kernel K[d,tau]=sum_n CB*A^tau, + D on the diagonal)
  - chunk-boundary states via a per-chunk increment matmul + a tiny 4-step recurrence
  - inter-chunk contribution via a CB*A^(i+1) matmul accumulated into the same PSUM
  - fused output projection with Wp.T

Sharding: 8 cores; core k handles batch b=k//2 and half=k%2:
  vertical  (scan over w): sequences (b, h in [32*half, 32*half+32))
  horizontal(scan over h): sequences (b, w in [32*half, 32*half+32))
Each core projects its two partial y tensors separately (projection is linear);
the host scatter-adds the two 2048-token contributions into the full output.
"""
import os
from contextlib import ExitStack

import numpy as np

import concourse.bass as bass
import concourse.bacc as bacc
import concourse.tile as tile
from concourse import mybir
from concourse.bass_utils import run_bass_kernel_spmd
from concourse.masks import make_identity

F32 = mybir.dt.float32
F16 = mybir.dt.float16
NP_CDT = np.float16
ALU = mybir.AluOpType
B, H, W, D, N = 4, 64, 64, 512, 8
L, Q, C, SEQ = 64, 16, 4, 32   # seq len, chunk size, n chunks, seqs/core/direction
NOCT = 64                      # octets of 8 channels
NG = 32                        # 2-octet groups


# ----------------------------------------------------------------------------
# host-side weight packing
# ----------------------------------------------------------------------------

def _precompute_weights(A, Bm, Cm, D_skip, Wp):
    A64, B64, C64 = A.astype(np.float64), Bm.astype(np.float64), Cm.astype(np.float64)
    CB = C64 * B64                                   # [D, N]
    Apow = np.stack([A64 ** t for t in range(Q + 1)])  # [Q+1, D, N]
    Kconv = np.einsum("dn,tdn->dt", CB, Apow)        # [D, Q+1]
    T = np.zeros((D, Q, Q))
    for i in range(Q):
        for j in range(i + 1):
            T[:, i, j] = Kconv[:, i - j]
    T += np.eye(Q)[None] * D_skip.astype(np.float64)[:, None, None]

    # K-rows ordered (j16, d8): row = j*8 + d8 (matches the PE-transpose output)
    W_T = np.zeros((NOCT, 128, 128))
    W_P = np.zeros((NOCT, 128, 64))
    for o in range(NOCT):
        for d8 in range(8):
            d = o * 8 + d8
            for j in range(Q):
                W_T[o, j * 8 + d8, d8::8] = T[d, :, j]
                W_P[o, j * 8 + d8, d8 * 8:d8 * 8 + 8] = Apow[Q - 1 - j, d]
    W_CBA = np.zeros((NG, 128, 256))
    for g in range(NG):
        for o2 in range(2):
            for d8 in range(8):
                d = g * 16 + o2 * 8 + d8
                for n in range(N):
                    row = o2 * 64 + d8 * 8 + n
                    W_CBA[g, row, o2 * 128 + d8:o2 * 128 + 128:8] = (
                        CB[d, n] * Apow[1:Q + 1, d, n]
                    )
    A16 = np.zeros((128, NG))
    for g in range(NG):
        for o2 in range(2):
            for d8 in range(8):
                d = g * 16 + o2 * 8 + d8
                A16[o2 * 64 + d8 * 8:o2 * 64 + d8 * 8 + 8, g] = Apow[Q, d]
    A16 = np.repeat(A16, SEQ, axis=1)  # [128, (g32, s32)]
    WPT = np.ascontiguousarray(Wp.astype(np.float64).T.reshape(4, 128, 512))
    return (W_T.astype(NP_CDT), W_P.astype(NP_CDT), W_CBA.astype(NP_CDT),
            A16.astype(np.float32), WPT.astype(NP_CDT))


# ----------------------------------------------------------------------------
# device program
# ----------------------------------------------------------------------------

def _one_direction(tc, pools, consts, x_view, z_view, tag, batched_out):
    """x_view: DRAM AP [c4, seq32, j16, d512]; z_view: [c4, seq32, i16, e512]."""
    nc = tc.nc
    (xnat_pool, xperm_pool, xt_pool, s_pool, y_pool, yt_pool, out_pool,
     psA, psyw, psG, psout) = pools
    w_t_sb, w_p_sb, w_cba_sb, a16_sb, wpt_sb, ident = consts

    x_nat = xnat_pool.tile([128, Q * D], F16, tag="xnat", name=f"xnat_{tag}")
    for c in range(C):
        nc.sync.dma_start(
            x_nat[c * SEQ:(c + 1) * SEQ, :].rearrange("s (j d) -> s j d", j=Q),
            x_view[c])
    # reorder (j16, d512) -> (o64, j16, d8): each octet one contiguous 128 block
    x_perm = xperm_pool.tile([128, Q * D], F16, tag="xperm", name=f"xperm_{tag}")
    x_src = x_nat[:].rearrange("p (j o e) -> p o j e", j=Q, o=NOCT, e=8)
    x_dst = x_perm[:].rearrange("p (o j e) -> p o j e", j=Q, o=NOCT, e=8)
    nc.vector.tensor_copy(x_dst[:, 0:32], x_src[:, 0:32])
    nc.vector.tensor_copy(x_dst[:, 32:64], x_src[:, 32:64])

    y_sb = y_pool.tile([128, NOCT * 128], F16, tag="y", name=f"y_{tag}")

    # phase T: transpose all octets into SBUF xt tiles (4 octets per tile)
    xts = []
    for og in range(16):
        ps_t = psA.tile([128, 512], F16, tag="ps_t")
        xt = xt_pool.tile([128, 512], F16, tag="xt")
        for oo in range(4):
            o = og * 4 + oo
            nc.tensor.transpose(
                ps_t[:, oo * 128:(oo + 1) * 128],
                x_perm[:, o * 128:(o + 1) * 128], ident)
        if og % 2 == 0:
            nc.scalar.copy(xt[:], ps_t[:])
        else:
            nc.vector.tensor_copy(xt[:], ps_t[:])
        xts.append(xt)

    def xt_oct(o):
        return xts[o // 4][:, (o % 4) * 128:(o % 4) * 128 + 128]

    # phase G: chunk-increment matmuls, 4 groups (8 octets) per PSUM bank,
    # then the batched 4-group chunk-state recurrence on DVE
    s_tiles = []
    for q in range(8):  # og-pairs: 8 octets each
        ps_g = psG.tile([128, 512], F32, tag="ps_g")
        for k in range(8):
            o = q * 8 + k
            half = (o % 2) * 64
            col = (k // 2) * 128
            nc.tensor.matmul(
                ps_g[half:half + 64, col:col + 128],
                w_p_sb[:, o * 64:o * 64 + 64], xt_oct(o),
                start=True, stop=True, skip_group_check=True,
                tile_position=(0, half))
        s4 = s_pool.tile([128, 512], F16, tag="s")
        sv = s4[:].rearrange("p (g c s) -> p g c s", g=4, c=C, s=SEQ)
        gv = ps_g[:].rearrange("p (g c s) -> p g c s", g=4, c=C, s=SEQ)
        a16b = a16_sb[:, q * 128:q * 128 + 128].rearrange(
            "p (g s) -> p g s", g=4)
        nc.gpsimd.memset(sv[:, :, 0, :], 0.0)
        nc.vector.tensor_copy(sv[:, :, 1, :], gv[:, :, 0, :])
        for cc in (2, 3):
            nc.vector.tensor_mul(sv[:, :, cc, :], sv[:, :, cc - 1, :], a16b)
            nc.vector.tensor_add(sv[:, :, cc, :], sv[:, :, cc, :], gv[:, :, cc - 1, :])
        s_tiles.append(s4)

    # phase B: intra-chunk matmuls + inter-chunk accumulation; 2 groups per bank
    for og in range(16):
        ps_yw = psyw.tile([128, 512], F32, tag="ps_yw")
        for oo in range(4):
            o = og * 4 + oo
            nc.tensor.matmul(ps_yw[:, oo * 128:oo * 128 + 128], xt_oct(o),
                             w_t_sb[:, o * 128:o * 128 + 128],
                             start=(oo == 0), stop=False, skip_group_check=True)
        for gg in range(2):
            g = og * 2 + gg
            s4 = s_tiles[g // 4]
            nc.tensor.matmul(ps_yw[:, gg * 256:gg * 256 + 256],
                             s4[:, (g % 4) * 128:(g % 4) * 128 + 128],
                             w_cba_sb[:, g * 256:g * 256 + 256],
                             start=False, stop=(gg == 1), skip_group_check=True)
        # scatter into y_sb layout (i16, o64, d8); ps_yw cols are (o4, i16, d8)
        y_dst = y_sb[:].rearrange("p (i og o e) -> p i og o e",
                                  i=Q, og=16, o=4, e=8)[:, :, og]
        ps_src = ps_yw[:].rearrange("p (o i e) -> p i o e", o=4, i=Q, e=8)
        if og % 2 == 0:
            nc.vector.tensor_copy(y_dst, ps_src)
        else:
            nc.scalar.copy(y_dst, ps_src)

    # projection: per i transpose y slice to [d, sc] then matmul with WpT
    for iq in range(4):
        out_sb = out_pool.tile([128, 4 * 512], F32, tag="osb")
        for ii in range(4):
            i = iq * 4 + ii
            ps_yt = psA.tile([128, 512], F16, tag="ps_t")
            for dc in range(4):
                nc.tensor.transpose(
                    ps_yt[:, dc * 128:dc * 128 + 128],
                    y_sb[:, i * 512 + dc * 128:i * 512 + (dc + 1) * 128], ident)
            yt = yt_pool.tile([128, 512], F16, tag="yt")
            if i % 2 == 0:
                nc.scalar.copy(yt[:], ps_yt[:])
            else:
                nc.vector.tensor_copy(yt[:], ps_yt[:])
            ps_o = psout.tile([128, 512], F32, tag="ps_o")
            for dc in range(4):
                nc.tensor.matmul(ps_o[:], yt[:, dc * 128:dc * 128 + 128],
                                 wpt_sb[:, dc * 512:dc * 512 + 512],
                                 start=(dc == 0), stop=(dc == 3))
            if i % 2 == 0:
                nc.vector.tensor_copy(out_sb[:, ii * 512:ii * 512 + 512], ps_o[:])
            else:
                nc.scalar.copy(out_sb[:, ii * 512:ii * 512 + 512], ps_o[:])
        if batched_out:
            nc.gpsimd.dma_start(z_view[:, :, iq * 4:iq * 4 + 4, :], out_sb[:])
        else:
            for ii in range(4):
                nc.gpsimd.dma_start(z_view[:, :, iq * 4 + ii, :],
                                  out_sb[:, ii * 512:ii * 512 + 512])


def _kernel_body(ctx, tc, aps):
    nc = tc.nc
    const_pool = ctx.enter_context(tc.tile_pool(name="consts", bufs=1))
    xnat_pool = ctx.enter_context(tc.tile_pool(name="xnat", bufs=2))
    xperm_pool = ctx.enter_context(tc.tile_pool(name="xperm", bufs=2))
    xt_pool = ctx.enter_context(tc.tile_pool(name="xt", bufs=20))
    s_pool = ctx.enter_context(tc.tile_pool(name="s", bufs=8))
    y_pool = ctx.enter_context(tc.tile_pool(name="y", bufs=2))
    yt_pool = ctx.enter_context(tc.tile_pool(name="yt", bufs=2))
    out_pool = ctx.enter_context(tc.tile_pool(name="osb", bufs=3))
    psA = ctx.enter_context(tc.tile_pool(name="psA", bufs=2, space="PSUM"))
    psyw = ctx.enter_context(tc.tile_pool(name="psyw", bufs=2, space="PSUM"))
    psG = ctx.enter_context(tc.tile_pool(name="psG", bufs=2, space="PSUM"))
    psout = ctx.enter_context(tc.tile_pool(name="psout", bufs=2, space="PSUM"))
    pools = (xnat_pool, xperm_pool, xt_pool, s_pool, y_pool, yt_pool, out_pool,
             psA, psyw, psG, psout)

    w_t_sb = const_pool.tile([128, NOCT * 128], F16, name="w_t_sb")
    w_p_sb = const_pool.tile([128, NOCT * 64], F16, name="w_p_sb")
    w_cba_sb = const_pool.tile([128, NG * 256], F16, name="w_cba_sb")
    a16_sb = const_pool.tile([128, NG * SEQ], F32, name="a16_sb")
    wpt_sb = const_pool.tile([128, 4 * 512], F16, name="wpt_sb")
    ident = const_pool.tile([128, 128], F16, name="ident")
    nc.scalar.dma_start(w_t_sb[:].rearrange("p (o m) -> p o m", o=NOCT),
                        aps["w_t"].rearrange("o p m -> p o m"))
    nc.scalar.dma_start(w_p_sb[:].rearrange("p (o m) -> p o m", o=NOCT),
                        aps["w_p"].rearrange("o p m -> p o m"))
    nc.scalar.dma_start(w_cba_sb[:].rearrange("p (g m) -> p g m", g=NG),
                        aps["w_cba"].rearrange("g p m -> p g m"))
    nc.scalar.dma_start(a16_sb[:], aps["a16"])
    nc.scalar.dma_start(wpt_sb[:].rearrange("p (c m) -> p c m", c=4),
                        aps["wpt"].rearrange("c p m -> p c m"))
    make_identity(nc, ident[:])
    consts = (w_t_sb[:], w_p_sb[:], w_cba_sb[:], a16_sb[:], wpt_sb[:], ident[:])

    # vertical: xv [32 (h-seq), 64 (w=pos), 512] ; zv same indexing
    xv_view = aps["xv"].rearrange("s (c j) d -> c s j d", c=C, j=Q)
    zv_view = aps["zv"].rearrange("s (c i) d -> c s i d", c=C, i=Q)
    _one_direction(tc, pools, consts, xv_view, zv_view, "v", True)
    # horizontal: xh [64 (h=pos), 32 (w-seq), 512]
    xh_view = aps["xh"].rearrange("(c j) s d -> c s j d", c=C, j=Q)
    zh_view = aps["zh"].rearrange("(c i) s d -> c s i d", c=C, i=Q)
    _one_direction(tc, pools, consts, xh_view, zh_view, "h", False)


def build_program(n_cores=8):
    nc = bacc.Bacc("TRN2", target_bir_lowering=False, debug=False,
                   enable_asserts=False, num_devices=n_cores)
    aps = {
        "xv": nc.dram_tensor("xv", [SEQ, L, D], F16, kind="ExternalInput").ap(),
        "xh": nc.dram_tensor("xh", [L, SEQ, D], F16, kind="ExternalInput").ap(),
        "w_t": nc.dram_tensor("w_t", [NOCT, 128, 128], F16, kind="ExternalInput").ap(),
        "w_p": nc.dram_tensor("w_p", [NOCT, 128, 64], F16, kind="ExternalInput").ap(),
        "w_cba": nc.dram_tensor("w_cba", [NG, 128, 256], F16, kind="ExternalInput").ap(),
        "a16": nc.dram_tensor("a16", [128, NG * SEQ], F32, kind="ExternalInput").ap(),
        "wpt": nc.dram_tensor("wpt", [4, 128, 512], F16, kind="ExternalInput").ap(),
        "zv": nc.dram_tensor("zv", [SEQ, L, D], F32, kind="ExternalOutput").ap(),
        "zh": nc.dram_tensor("zh", [L, SEQ, D], F32, kind="ExternalOutput").ap(),
    }
    with tile.TileContext(nc) as tc:
        with ExitStack() as ctx:
            _kernel_body(ctx, tc, aps)
    nc.compile()
    return nc


_PROGRAM = None


def _get_program():
    global _PROGRAM
    if _PROGRAM is None:
        _PROGRAM = build_program()
    return _PROGRAM


def make_in_maps(x, A, Bm, Cm, D_skip, Wp):
    W_T, W_P, W_CBA, A16, WPT = _precompute_weights(A, Bm, Cm, D_skip, Wp)
    xg = np.ascontiguousarray(x, dtype=np.float32).reshape(B, H, W, D)
    in_maps = []
    for k in range(8):
        b, half = k // 2, k % 2
        in_maps.append({
            "xv": np.ascontiguousarray(xg[b, 32 * half:32 * half + 32]).astype(NP_CDT),
            "xh": np.ascontiguousarray(xg[b, :, 32 * half:32 * half + 32]).astype(NP_CDT),
            "w_t": W_T, "w_p": W_P, "w_cba": W_CBA, "a16": A16, "wpt": WPT,
        })
    return in_maps


def assemble_output(results, b_proj):
    out = np.zeros((B, H, W, D), np.float32)
    for k in range(8):
        b, half = k // 2, k % 2
        out[b, 32 * half:32 * half + 32, :, :] += results[k]["zv"]
        out[b, :, 32 * half:32 * half + 32, :] += results[k]["zh"]
    out += np.asarray(b_proj, dtype=np.float32)
    return out.reshape(B, H * W, D)


def kernel(x, h, w, A, Bm, Cm, D_skip, Wp, b_proj, **_kw):
    nc = _get_program()
    in_maps = make_in_maps(np.asarray(x), np.asarray(A), np.asarray(Bm),
                           np.asarray(Cm), np.asarray(D_skip), np.asarray(Wp))
    res = run_bass_kernel_spmd(nc, in_maps, list(range(8)))
    return assemble_output(res.results, np.asarray(b_proj))



# revision 4
# speedup vs baseline: 1.3350x; 1.3350x over previous
"""Trainium2 Bass kernel for nn_DirectionalScan (2D directional diagonal-SSM + projection).

Math: for each of two directions (scan over h, scan over w),
    y[t] = sum_n Cm*Bm * sum_{u<=t} A^(t-u) x[u]  + D_skip*x[t]
then out = (y_h + y_v) @ Wp.T + b_proj.

Device decomposition: chunked SSM with chunk Q=16, all heavy work on the PE:
  - intra-chunk causal Toeplitz (kernel K[d,tau]=sum_n CB*A^tau, + D on the diagonal)
  - chunk-boundary states via a per-chunk increment matmul + a batched 3-step recurrence
  - inter-chunk contribution via a CB*A^(i+1) matmul accumulated into the same PSUM
  - fused output projection with Wp.T

Layout strategy (v2): the host pre-packs x into the PE-transposed
(j16,e8)-partition layout and all weights into SBUF-native [128, X] images, so
every load DMA is fully contiguous and the on-device transpose/permute phase of
v1 disappears.  Outputs are written in fp16 in the SBUF-native token-chunk
layout through the two hardware DGE queues; the host unpacks and scatter-adds.

Sharding: 8 cores; core k handles batch b=k//2 and half=k%2:
  vertical  (scan over w): sequences (b, h in [32*half, 32*half+32))
  horizontal(scan over h): sequences (b, w in [32*half, 32*half+32))
Each core projects its two partial y tensors separately (projection is linear);
the host scatter-adds the two 2048-token contributions into the full output.
"""
import os
from contextlib import ExitStack

import numpy as np

import concourse.bass as bass
import concourse.bacc as bacc
import concourse.tile as tile
from concourse import mybir
from concourse.bass_utils import run_bass_kernel_spmd
from concourse.masks import make_identity

F32 = mybir.dt.float32
F16 = mybir.dt.float16
NP_CDT = np.float16
B, H, W, D, N = 4, 64, 64, 512, 8
L, Q, C, SEQ = 64, 16, 4, 32   # seq len, chunk size, n chunks, seqs/core/direction
NOCT = 64                      # octets of 8 channels
NG = 32                        # 2-octet groups


# ----------------------------------------------------------------------------
# host-side packing
# ----------------------------------------------------------------------------

def _precompute_weights(A, Bm, Cm, D_skip, Wp):
    A64, B64, C64 = A.astype(np.float64), Bm.astype(np.float64), Cm.astype(np.float64)
    CB = C64 * B64                                   # [D, N]
    Apow = np.stack([A64 ** t for t in range(Q + 1)])  # [Q+1, D, N]
    Kconv = np.einsum("dn,tdn->dt", CB, Apow)        # [D, Q+1]
    T = np.zeros((D, Q, Q))
    for i in range(Q):
        for j in range(i + 1):
            T[:, i, j] = Kconv[:, i - j]
    T += np.eye(Q)[None] * D_skip.astype(np.float64)[:, None, None]

    # K-rows ordered (j16, d8): row = j*8 + d8
    W_T = np.zeros((128, NOCT, 128))
    W_P = np.zeros((128, NOCT, 64))
    for o in range(NOCT):
        for d8 in range(8):
            d = o * 8 + d8
            for j in range(Q):
                W_T[j * 8 + d8, o, d8::8] = T[d, :, j]
                W_P[j * 8 + d8, o, d8 * 8:d8 * 8 + 8] = Apow[Q - 1 - j, d]
    W_CBA = np.zeros((128, NG, 256))
    for g in range(NG):
        for o2 in range(2):
            for d8 in range(8):
                d = g * 16 + o2 * 8 + d8
                for n in range(N):
                    row = o2 * 64 + d8 * 8 + n
                    W_CBA[row, g, o2 * 128 + d8:o2 * 128 + 128:8] = (
                        CB[d, n] * Apow[1:Q + 1, d, n]
                    )
    A16 = np.zeros((128, NG))
    for g in range(NG):
        for o2 in range(2):
            for d8 in range(8):
                d = g * 16 + o2 * 8 + d8
                A16[o2 * 64 + d8 * 8:o2 * 64 + d8 * 8 + 8, g] = Apow[Q, d]
    A16 = np.repeat(A16, SEQ, axis=1)  # [128, (g32, s32)]
    # WPT[p, dc*512+dout] = Wp[dout, dc*128+p]
    WPT = np.ascontiguousarray(
        Wp.astype(np.float64).T.reshape(4, 128, D).transpose(1, 0, 2).reshape(128, 4 * D))
    return (W_T.reshape(128, NOCT * 128).astype(NP_CDT),
            W_P.reshape(128, NOCT * 64).astype(NP_CDT),
            W_CBA.reshape(128, NG * 256).astype(NP_CDT),
            A16.astype(np.float32), WPT.astype(NP_CDT))


def _pack_xt(x_dir):
    """x_dir [32 seq, 64 pos, 512 d] -> XT [128=(j16,e8), 64 oct * 128=(c4,s32)]."""
    v = x_dir.reshape(SEQ, C, Q, NOCT, 8)            # s c j o e
    v = v.transpose(2, 4, 3, 1, 0)                   # j e o c s
    return np.ascontiguousarray(v.reshape(128, NOCT * 128), dtype=NP_CDT)


# ----------------------------------------------------------------------------
# device program
# ----------------------------------------------------------------------------

def _one_direction(tc, pools, consts, xt_dram, z_dram, tag, first):
    nc = tc.nc
    (xt_pool, g_pool, s_pool, y_pool, yt_pool, out_pool,
     psA, psyw, psG, psout) = pools
    w_t_sb, w_p_sb, w_cba_sb, a16_sb, wpt_sb, ident = consts

    xt = xt_pool.tile([128, NOCT * 128], F16, tag="xt", name=f"xt_{tag}")
    nc.sync.dma_start(xt[:, :4096], xt_dram[:, :4096])
    nc.sync.dma_start(xt[:, 4096:], xt_dram[:, 4096:])

    def xt_oct(o):
        return xt[:, o * 128:(o + 1) * 128]

    if first:
        # warm the PE HAM clock gate while the first x tile loads
        ps_w = psG.tile([128, 512], F16, tag="ps_g", name="warm")
        for _ in range(10):
            for j in range(4):
                nc.tensor.transpose(ps_w[:, j * 128:(j + 1) * 128], ident, ident)

    # phase G: per-chunk state increments, 8 octets per PSUM bank
    g_all = g_pool.tile([128, 8 * 512], F16, tag="g", name=f"g_{tag}")
    for q in range(8):
        ps_g = psG.tile([128, 512], F32, tag="ps_g")
        for k in range(8):
            o = q * 8 + k
            half = (o % 2) * 64
            col = (k // 2) * 128
            nc.tensor.matmul(
                ps_g[half:half + 64, col:col + 128],
                w_p_sb[:, o * 64:o * 64 + 64], xt_oct(o),
                start=True, stop=True, skip_group_check=True,
                tile_position=(0, half))
        if q % 2 == 0:
            nc.scalar.copy(g_all[:, q * 512:(q + 1) * 512], ps_g[:])
        else:
            nc.vector.tensor_copy(g_all[:, q * 512:(q + 1) * 512], ps_g[:])

    # batched chunk-state recurrence over all 32 (q,g) groups at once
    s_all = s_pool.tile([128, NG * 128], F16, tag="s", name=f"s_{tag}")
    sv = s_all[:].rearrange("p (g c s) -> p g c s", g=NG, c=C, s=SEQ)
    gv = g_all[:].rearrange("p (g c s) -> p g c s", g=NG, c=C, s=SEQ)
    av = a16_sb.rearrange("p (g s) -> p g s", g=NG)
    nc.gpsimd.memset(sv[:, :, 0, :], 0.0)
    nc.vector.tensor_copy(sv[:, :, 1, :], gv[:, :, 0, :])
    for cc in (2, 3):
        nc.vector.tensor_mul(sv[:, :, cc, :], sv[:, :, cc - 1, :], av)
        nc.vector.tensor_add(sv[:, :, cc, :], sv[:, :, cc, :], gv[:, :, cc - 1, :])

    # phase B: intra-chunk Toeplitz + inter-chunk accumulation; 1 bank per 4 octets
    y_sb = y_pool.tile([128, NOCT * 128], F16, tag="y", name=f"y_{tag}")
    for og in range(16):
        ps_yw = psyw.tile([128, 512], F32, tag="ps_yw")
        for oo in range(4):
            o = og * 4 + oo
            nc.tensor.matmul(ps_yw[:, oo * 128:(oo + 1) * 128], xt_oct(o),
                             w_t_sb[:, o * 128:(o + 1) * 128],
                             start=(oo == 0), stop=False, skip_group_check=True)
        for gg in range(2):
            g = og * 2 + gg
            nc.tensor.matmul(ps_yw[:, gg * 256:(gg + 1) * 256],
                             s_all[:, g * 128:(g + 1) * 128],
                             w_cba_sb[:, g * 256:(g + 1) * 256],
                             start=False, stop=(gg == 1), skip_group_check=True)
        # scatter into y_sb layout (i16, d512); ps_yw cols are (oo4, i16, e8)
        y_dst = y_sb[:].rearrange("p (i og oo e) -> p og i oo e",
                                  i=Q, og=16, oo=4, e=8)[:, og]
        ps_src = ps_yw[:].rearrange("p (oo i e) -> p i oo e", oo=4, i=Q, e=8)
        if og % 2 == 0:
            nc.vector.tensor_copy(y_dst, ps_src)
        else:
            nc.scalar.copy(y_dst, ps_src)

    # projection: per i, transpose y cols for pos i to [d, token] then matmul WpT
    # y_sb col = i*512 + og*32 + oo*8 + e; global d = og*32 + oo*8 + e
    for iq in range(4):
        out_sb = out_pool.tile([128, 4 * 512], F16, tag="osb")
        for ii in range(4):
            i = iq * 4 + ii
            ps_yt = psA.tile([128, 512], F16, tag="ps_t")
            for dc in range(4):
                nc.tensor.transpose(
                    ps_yt[:, dc * 128:(dc + 1) * 128],
                    y_sb[:, i * 512 + dc * 128:i * 512 + (dc + 1) * 128], ident)
            yt = yt_pool.tile([128, 512], F16, tag="yt")
            if i % 2 == 0:
                nc.scalar.copy(yt[:], ps_yt[:])
            else:
                nc.vector.tensor_copy(yt[:], ps_yt[:])
            ps_o = psout.tile([128, 512], F32, tag="ps_o")
            for dc in range(4):
                nc.tensor.matmul(ps_o[:], yt[:, dc * 128:(dc + 1) * 128],
                                 wpt_sb[:, dc * 512:(dc + 1) * 512],
                                 start=(dc == 0), stop=(dc == 3))
            if i % 2 == 0:
                nc.vector.tensor_copy(out_sb[:, ii * 512:(ii + 1) * 512], ps_o[:])
            else:
                nc.scalar.copy(out_sb[:, ii * 512:(ii + 1) * 512], ps_o[:])
        if iq % 2 == 0:
            nc.sync.dma_start(z_dram[:, iq * 2048:(iq + 1) * 2048], out_sb[:])
        else:
            nc.scalar.dma_start(z_dram[:, iq * 2048:(iq + 1) * 2048], out_sb[:])


def _kernel_body(ctx, tc, aps):
    nc = tc.nc
    const_pool = ctx.enter_context(tc.tile_pool(name="consts", bufs=1))
    xt_pool = ctx.enter_context(tc.tile_pool(name="xt", bufs=2))
    g_pool = ctx.enter_context(tc.tile_pool(name="g", bufs=2))
    s_pool = ctx.enter_context(tc.tile_pool(name="s", bufs=2))
    y_pool = ctx.enter_context(tc.tile_pool(name="y", bufs=2))
    yt_pool = ctx.enter_context(tc.tile_pool(name="yt", bufs=3))
    out_pool = ctx.enter_context(tc.tile_pool(name="osb", bufs=3))
    psA = ctx.enter_context(tc.tile_pool(name="psA", bufs=2, space="PSUM"))
    psyw = ctx.enter_context(tc.tile_pool(name="psyw", bufs=2, space="PSUM"))
    psG = ctx.enter_context(tc.tile_pool(name="psG", bufs=2, space="PSUM"))
    psout = ctx.enter_context(tc.tile_pool(name="psout", bufs=2, space="PSUM"))
    pools = (xt_pool, g_pool, s_pool, y_pool, yt_pool, out_pool,
             psA, psyw, psG, psout)

    w_p_sb = const_pool.tile([128, NOCT * 64], F16, name="w_p_sb")
    a16_sb = const_pool.tile([128, NG * SEQ], F32, name="a16_sb")
    w_t_sb = const_pool.tile([128, NOCT * 128], F16, name="w_t_sb")
    w_cba_sb = const_pool.tile([128, NG * 256], F16, name="w_cba_sb")
    wpt_sb = const_pool.tile([128, 4 * 512], F16, name="wpt_sb")
    ident = const_pool.tile([128, 128], F16, name="ident")
    make_identity(nc, ident[:])
    nc.scalar.dma_start(w_p_sb[:], aps["w_p"])
    nc.scalar.dma_start(a16_sb[:], aps["a16"])
    nc.scalar.dma_start(w_t_sb[:], aps["w_t"])
    nc.scalar.dma_start(w_cba_sb[:], aps["w_cba"])
    nc.scalar.dma_start(wpt_sb[:], aps["wpt"])
    consts = (w_t_sb[:], w_p_sb[:], w_cba_sb[:], a16_sb[:], wpt_sb[:], ident[:])

    _one_direction(tc, pools, consts, aps["xv"], aps["zv"], "v", True)
    _one_direction(tc, pools, consts, aps["xh"], aps["zh"], "h", False)


def build_program(n_cores=8):
    nc = bacc.Bacc("TRN2", target_bir_lowering=False, debug=False,
                   enable_asserts=False, num_devices=n_cores)
    aps = {
        "xv": nc.dram_tensor("xv", [128, NOCT * 128], F16, kind="ExternalInput").ap(),
        "xh": nc.dram_tensor("xh", [128, NOCT * 128], F16, kind="ExternalInput").ap(),
        "w_t": nc.dram_tensor("w_t", [128, NOCT * 128], F16, kind="ExternalInput").ap(),
        "w_p": nc.dram_tensor("w_p", [128, NOCT * 64], F16, kind="ExternalInput").ap(),
        "w_cba": nc.dram_tensor("w_cba", [128, NG * 256], F16, kind="ExternalInput").ap(),
        "a16": nc.dram_tensor("a16", [128, NG * SEQ], F32, kind="ExternalInput").ap(),
        "wpt": nc.dram_tensor("wpt", [128, 4 * 512], F16, kind="ExternalInput").ap(),
        "zv": nc.dram_tensor("zv", [128, Q * 512], F16, kind="ExternalOutput").ap(),
        "zh": nc.dram_tensor("zh", [128, Q * 512], F16, kind="ExternalOutput").ap(),
    }
    with tile.TileContext(nc) as tc:
        with ExitStack() as ctx:
            _kernel_body(ctx, tc, aps)
    nc.compile()
    return nc


_PROGRAM = None


def _get_program():
    global _PROGRAM
    if _PROGRAM is None:
        _PROGRAM = build_program()
    return _PROGRAM


def make_in_maps(x, A, Bm, Cm, D_skip, Wp):
    W_T, W_P, W_CBA, A16, WPT = _precompute_weights(A, Bm, Cm, D_skip, Wp)
    xg = np.ascontiguousarray(x, dtype=np.float32).reshape(B, H, W, D)
    in_maps = []
    for k in range(8):
        b, half = k // 2, k % 2
        xv = np.ascontiguousarray(xg[b, 32 * half:32 * half + 32])      # [32 h, 64 w, d]
        xh = np.ascontiguousarray(
            xg[b, :, 32 * half:32 * half + 32].transpose(1, 0, 2))     # [32 w, 64 h, d]
        in_maps.append({
            "xv": _pack_xt(xv), "xh": _pack_xt(xh),
            "w_t": W_T, "w_p": W_P, "w_cba": W_CBA, "a16": A16, "wpt": WPT,
        })
    return in_maps


def assemble_output(results, b_proj):
    out = np.zeros((B, H, W, D), np.float32)
    for k in range(8):
        b, half = k // 2, k % 2
        # z [128=(c4,s32), (i16, d512)] -> [s, c*16+i, d]
        zv = results[k]["zv"].astype(np.float32).reshape(C, SEQ, Q, D)
        zh = results[k]["zh"].astype(np.float32).reshape(C, SEQ, Q, D)
        out[b, 32 * half:32 * half + 32, :, :] += zv.transpose(1, 0, 2, 3).reshape(SEQ, L, D)
        out[b, :, 32 * half:32 * half + 32, :] += zh.transpose(0, 2, 1, 3).reshape(L, SEQ, D)
    out += np.asarray(b_proj, dtype=np.float32)
    return out.reshape(B, H * W, D)


def kernel(x, h, w, A, Bm, Cm, D_skip, Wp, b_proj, **_kw):
    nc = _get_program()
    in_maps = make_in_maps(np.asarray(x), np.asarray(A), np.asarray(Bm),
                           np.asarray(Cm), np.asarray(D_skip), np.asarray(Wp))
    res = run_bass_kernel_spmd(nc, in_maps, list(range(8)))
    return assemble_output(res.results, np.asarray(b_proj))
